# revision 35
# baseline (speedup 1.0000x reference)
"""Trainium2 Bass kernel for nn_MultiHeadAttention_824633721543.

MHA with periodic prefix mask: allowed iff (q % 256) >= (k % 256).
B=2, S=2048, D=768, H=12, Dk=64, WINDOW=256.

Sharding: 8 cores = 2 batches x 4 head-groups (3 heads each). Each core
computes q/k/v projections for its heads, the masked softmax attention, and
a partial O-projection; the host sums the 4 partials per batch and adds bo.

Device-side layout (all transpose-free, fp16 matmul datapath, f32 PSUM):
  - scores computed as S^T [k,q]: kT slice stationary, qT moving
  - q columns tile-permuted (even 128-tiles | odd 128-tiles) so the mask is:
      even-group x k-lo  -> one shared 128x128 triu tile (0/1 multiply)
      odd-group  x k-lo  -> unmasked
      odd-group  x k-hi  -> shared triu
      even-group x k-hi  -> fully masked, never computed
  - exp on ACT (scale=1/8 folded in, f32 psum in -> fp16 out), no
    max-subtraction (scores are small)
  - P@V with [V|1] stationary -> out^T plus denominator row, accumulated over
    the 8 windows in PSUM; normalization: reciprocal of the denom row,
    K=1 broadcast matmul, DVE multiply
  - O-projection: h0+h1 stacked into one K=128 stationary + h2 at K=64;
    result DMA'd to DRAM directly from PSUM (f32)
"""

import sys

sys.path.insert(0, "/opt/trn_rl_repo")

import numpy as np
import ml_dtypes

bf16 = np.float16

B = 2
S = 2048
D = 768
DK = 64
WIN = 256
NW = S // WIN   # 8 windows
NHC = 3         # heads per core
DH = NHC * DK   # 192
NT = S // 128   # 16 q tiles

_CACHE = {}

# v-tile init pattern: three 128-col groups, ones at col 64 of each group
_VINIT = np.zeros((128, 384), np.float16)
_VINIT[:, [64, 192, 320]] = 1.0


def _build_program():
    import concourse.tile as tile
    from concourse import mybir, bacc
    from contextlib import ExitStack

    f32 = mybir.dt.float32
    f16 = mybir.dt.float16
    Exp = mybir.ActivationFunctionType.Exp
    Ident = mybir.ActivationFunctionType.Identity
    mult = mybir.AluOpType.mult
    add = mybir.AluOpType.add

    nc = bacc.Bacc("TRN2", target_bir_lowering=False, debug=False)

    xT = nc.dram_tensor("xT", [D, S], f16, kind="ExternalInput").ap()
    w1 = nc.dram_tensor("w1", [D, 256], f16, kind="ExternalInput").ap()  # [qh01|kh01]
    w2 = nc.dram_tensor("w2", [D, 128], f16, kind="ExternalInput").ap()  # [qh2|kh2]
    wv = nc.dram_tensor("wv", [D, 192], f16, kind="ExternalInput").ap()  # WvT
    wo = nc.dram_tensor("wo", [256, D], f16, kind="ExternalInput").ap()
    btA = nc.dram_tensor("btA", [128, 1], f32, kind="ExternalInput").ap()
    btB = nc.dram_tensor("btB", [128, 1], f32, kind="ExternalInput").ap()
    btCD = nc.dram_tensor("btCD", [128, 1], f32, kind="ExternalInput").ap()
    bvb = nc.dram_tensor("bvb", [128, 192], f32, kind="ExternalInput").ap()
    stair = nc.dram_tensor("stair", [128, 128], f16, kind="ExternalInput").ap()
    zq = nc.dram_tensor("zq", [64, S], f16, kind="ExternalInput").ap()
    vinit = nc.dram_tensor("vinit", [128, 384], f16, kind="ExternalInput").ap()
    onesc = nc.dram_tensor("onesc", [65, 64], f16, kind="ExternalInput").ap()
    out = nc.dram_tensor("out", [S, D], f16, kind="ExternalOutput").ap()
    import os as _os
    DBG = bool(_os.environ.get("MHA_DEBUG"))
    if DBG:
        dbgA = nc.dram_tensor("dbgA", [128, S], f16, kind="ExternalOutput").ap()
        dbg2 = nc.dram_tensor("dbg2", [64, S], f16, kind="ExternalOutput").ap()
        dbgden = nc.dram_tensor("dbgden", [192, S], f32, kind="ExternalOutput").ap()

    with tile.TileContext(nc) as tc, ExitStack() as ctx:
        consts = ctx.enter_context(tc.tile_pool(name="consts", bufs=1))
        qkv = ctx.enter_context(tc.tile_pool(name="qkv", bufs=1))

        xT_sb = [qkv.tile([128, S], f16, tag=f"xt{k}", name=f"xt{k}")
                 for k in range(6)]
        w1_sb = [consts.tile([128, 256], f16, tag=f"w1_{k}", name=f"w1s{k}")
                 for k in range(6)]
        w2_sb = [consts.tile([128, 128], f16, tag=f"w2_{k}", name=f"w2s{k}")
                 for k in range(6)]
        wv_sb = [consts.tile([128, 192], f16, tag=f"wv_{k}", name=f"wvs{k}")
                 for k in range(6)]
        for k in range(6):
            nc.sync.dma_start(out=xT_sb[k], in_=xT[k * 128:(k + 1) * 128, :])
            nc.sync.dma_start(out=w1_sb[k], in_=w1[k * 128:(k + 1) * 128, :])
            nc.sync.dma_start(out=w2_sb[k], in_=w2[k * 128:(k + 1) * 128, :])
            nc.sync.dma_start(out=wv_sb[k], in_=wv[k * 128:(k + 1) * 128, :])
        # Wo^T: h0+h1 as one 128-row tile, h2 zero-padded to 128 rows
        wo01_sb = consts.tile([128, D], f16, tag="wo01")
        wo2_sb = consts.tile([128, D], f16, tag="wo2")
        nc.sync.dma_start(out=wo01_sb, in_=wo[0:128, :])
        nc.sync.dma_start(out=wo2_sb, in_=wo[128:256, :])
        btA_sb = consts.tile([128, 1], f32, tag="btA")
        btB_sb = consts.tile([128, 1], f32, tag="btB")
        btCD_sb = consts.tile([128, 1], f32, tag="btCD")
        nc.sync.dma_start(out=btA_sb, in_=btA)
        nc.sync.dma_start(out=btB_sb, in_=btB)
        nc.sync.dma_start(out=btCD_sb, in_=btCD)
        bvb_sb = consts.tile([128, 192], f32, tag="bvb")
        nc.sync.dma_start(out=bvb_sb, in_=bvb)
        stair_sb = consts.tile([128, 128], f16, tag="stair")
        nc.sync.dma_start(out=stair_sb, in_=stair)
        ones_row = consts.tile([65, 64], f16, tag="ones_row")
        nc.sync.dma_start(out=ones_row, in_=onesc)

        # ---- long-lived activation tiles ----
        # Per-head qT/kT tiles [128, S]: K=128 stationaries for the PE.
        # h0/h2 data lives in rows 0:64, h1/kh2 in rows 64:128; the other
        # 64 rows are zero so the padded contraction adds nothing.
        qh = [qkv.tile([128, S], f16, tag=f"qh{i}", name=f"qh{i}")
              for i in range(3)]
        kh = [qkv.tile([128, S], f16, tag=f"kh{i}", name=f"kh{i}")
              for i in range(3)]
        qrow = [0, 64, 64]  # base row of the 64 data rows per head
        krow = [0, 64, 64]
        for i in range(3):
            zr = 64 - qrow[i]
            nc.sync.dma_start(out=qh[i][zr:zr + 64, :], in_=zq)
            zr = 64 - krow[i]
            nc.sync.dma_start(out=kh[i][zr:zr + 64, :], in_=zq)
        # v natural [s,d] per s-tile: three 128-col groups [V_h | 1 | 0...]
        v_sb = [qkv.tile([128, 384], f16, tag=f"v{i}", name=f"vsb{i}")
                for i in range(NT)]
        for st in range(NT):
            nc.sync.dma_start(out=v_sb[st], in_=vinit)
        # attn^T: h0 rows 0:64 + h1 rows 64:128 in one tile; h2 zero-padded
        attnT01 = qkv.tile([128, S], f16, tag="attnT01")
        attnT2 = qkv.tile([128, S], f16, tag="attnT2")
        nc.sync.dma_start(out=attnT2[64:128, :], in_=zq)

        def permuted_copy(dst, r0, ps, n, bias, eng, src_r0=None):
            """psum rows src_r0:+64, 512-span n -> dst rows r0:r0+64 with
            even/odd q-tile permutation."""
            s0 = r0 if src_r0 is None else src_r0
            pr3 = ps[s0:s0 + 64, :].rearrange(
                "p (c two k) -> p c two k", two=2, k=128)
            dr = dst[r0:r0 + 64, :]
            bs = bias[s0:s0 + 64, :]
            eng.tensor_scalar_add(
                out=dr[:, 256 * n:256 * n + 256].rearrange("p (c k) -> p c k", k=128),
                in0=pr3[:, :, 0, :], scalar1=bs)
            eng.tensor_scalar_add(
                out=dr[:, 1024 + 256 * n:1024 + 256 * n + 256].rearrange(
                    "p (c k) -> p c k", k=128),
                in0=pr3[:, :, 1, :], scalar1=bs)

        # ---- stage A ----
        with tc.tile_pool(name="psA", bufs=2, space="PSUM") as psA:
            for n in range(4):
                xn = [xT_sb[k][:, 512 * n:512 * (n + 1)]
                      for k in range(6)]
                psa = psA.tile([128, 512], f32, tag="psA")
                for k in range(6):
                    nc.tensor.matmul(psa, w1_sb[k][:, 0:128], xn[k],
                                     start=(k == 0), stop=(k == 5))
                permuted_copy(qh[0], 0, psa, n, btA_sb, nc.vector)
                permuted_copy(qh[1], 64, psa, n, btA_sb, nc.vector)
                psb = psA.tile([128, 512], f32, tag="psA")
                for k in range(6):
                    nc.tensor.matmul(psb, w1_sb[k][:, 128:256], xn[k],
                                     start=(k == 0), stop=(k == 5))
                nc.scalar.activation(
                    out=kh[0][0:64, 512 * n:512 * (n + 1)], in_=psb[0:64, :],
                    func=Ident, bias=btB_sb[0:64, :])
                nc.scalar.activation(
                    out=kh[1][64:128, 512 * n:512 * (n + 1)], in_=psb[64:128, :],
                    func=Ident, bias=btB_sb[64:128, :])
                psqk = psA.tile([128, 512], f32, tag="psA")
                for k in range(6):
                    nc.tensor.matmul(psqk, w2_sb[k], xn[k],
                                     start=(k == 0), stop=(k == 5))
                permuted_copy(qh[2], 64, psqk, n, btCD_sb, nc.vector,
                              src_r0=0)
                nc.scalar.activation(
                    out=kh[2][64:128, 512 * n:512 * (n + 1)], in_=psqk[64:128, :],
                    func=Ident, bias=btCD_sb[64:128, :])


        # ---- stage B ----
        heads = [
            dict(q=qh[0], k=kh[0], o=(attnT01, 0)),
            dict(q=qh[1], k=kh[1], o=(attnT01, 64)),
            dict(q=qh[2], k=kh[2], o=(attnT2, 0)),
        ]

        stair_b = stair_sb.unsqueeze(1).broadcast_to([128, 8, 128])
        with tc.tile_pool(name="pt", bufs=10) as pt_pool, \
             tc.tile_pool(name="sc", bufs=3, space="PSUM") as sc_pool, \
             tc.tile_pool(name="po", bufs=2, space="PSUM") as out_pool, \
             tc.tile_pool(name="psv", bufs=1, space="PSUM") as psv_pool, \
             tc.tile_pool(name="nrm", bufs=2) as nrm_pool:
            def v_proj(st):
                psv = psv_pool.tile([128, 192], f32, tag="psv")
                for k in range(6):
                    nc.tensor.matmul(
                        psv, xT_sb[k][:, 128 * st:128 * (st + 1)],
                        wv_sb[k], start=(k == 0), stop=(k == 5))
                vt = v_sb[st]
                nc.vector.tensor_tensor(
                    out=vt.rearrange("p (h c) -> p h c", c=128)[:, :, 0:64],
                    in0=psv.rearrange("p (h c) -> p h c", c=64),
                    in1=bvb_sb.rearrange("p (h c) -> p h c", c=64), op=add)

            for h in range(NHC):
                hd = heads[h]
                qv = hd["q"]
                kv = hd["k"]
                ot, ooff = hd["o"]
                vh = [v[:, 128 * h:128 * (h + 1)] for v in v_sb]  # [V_h|1|0]

                po = [out_pool.tile([128, 1024], f32, tag="po",
                                    name=f"po{h}_{g}") for g in range(2)]
                first = [[True, True], [True, True]]

                def pv_mm(g, vsl, pt, last):
                    for sub in range(2):
                        nc.tensor.matmul(
                            po[g][:, 512 * sub:512 * (sub + 1)],
                            vsl,
                            pt[:, 512 * sub:512 * (sub + 1)],
                            start=first[g][sub], stop=last)
                        first[g][sub] = False

                def scores_exp(g, kblk, mask):
                    qcols = qv[:, 1024 * g:1024 * (g + 1)]
                    pt = pt_pool.tile([128, 1024], f16, tag="pt")
                    for sub in range(2):
                        sch = sc_pool.tile([128, 512], f32, tag="sc")
                        nc.tensor.matmul(
                            sch, kblk, qcols[:, 512 * sub:512 * (sub + 1)],
                            start=True, stop=True)
                        nc.scalar.activation(
                            out=pt[:, 512 * sub:512 * (sub + 1)], in_=sch,
                            func=Exp, scale=0.125)
                    if mask:
                        p3 = pt.rearrange("p (c k) -> p c k", k=128)
                        nc.vector.tensor_mul(out=p3, in0=p3, in1=stair_b)
                    return pt

                # g0/g1 interleaved, PV one window behind scores
                pend = None
                for w in range(NW + 1):
                    if w < NW:
                        if h == 0:
                            v_proj(2 * w)
                            v_proj(2 * w + 1)
                        klo = kv[:, WIN * w:WIN * w + 128]
                        khi = kv[:, WIN * w + 128:WIN * w + 256]
                        cur = (scores_exp(0, klo, True),
                               scores_exp(1, klo, False),
                               scores_exp(1, khi, True))
                    if pend is not None:
                        pw = w - 1
                        last = (pw == NW - 1)
                        pv_mm(0, vh[2 * pw], pend[0], last)
                        pv_mm(1, vh[2 * pw], pend[1], False)
                        pv_mm(1, vh[2 * pw + 1], pend[2], last)
                    pend = cur if w < NW else None

                # normalization: reciprocal of denom row 64 -> bcast -> mul
                for g in range(2):
                    poc = nrm_pool.tile([65, 1024], f16, tag="poc",
                                        name=f"poc{h}{g}")
                    nc.vector.tensor_copy(out=poc, in_=po[g][0:65, :])
                    rec_sb = nrm_pool.tile([64, 1024], f32, tag="rec")
                    for sub in range(2):
                        rec_ps = sc_pool.tile([128, 512], f32, tag="sc")
                        nc.tensor.matmul(
                            rec_ps[0:64, :],
                            ones_row[64:65, :],
                            poc[64:65, 512 * sub:512 * (sub + 1)],
                            start=True, stop=True)
                        nc.vector.reciprocal_approx_fast(
                            out=rec_sb[:, 512 * sub:512 * (sub + 1)],
                            in_=rec_ps[0:64, :])
                    nc.vector.tensor_tensor(
                        out=ot[ooff:ooff + 64, 1024 * g:1024 * (g + 1)],
                        in0=poc[0:64, :], in1=rec_sb, op=mult)
                    if DBG:
                        dent = nrm_pool.tile([128, 1024], f32, tag=f"dent{h}{g}",
                                             name=f"dent{h}{g}")
                        nc.vector.tensor_copy(out=dent[0:64, :], in_=rec_ps[0:64, :])
                        nc.sync.dma_start(
                            out=dbgden[64 * h:64 * (h + 1), 1024 * g:1024 * (g + 1)],
                            in_=dent[0:64, :])

        if DBG:
            nc.sync.dma_start(out=dbgA, in_=attnT01)
            nc.sync.dma_start(out=dbg2, in_=attnT2)

        # ---- stage C ----
        with tc.tile_pool(name="oc", bufs=4, space="PSUM") as oc_pool, \
             tc.tile_pool(name="ost", bufs=4) as ost_pool:
            for p in range(NT):
                pso = oc_pool.tile([128, D], f32, tag="pso")
                for (n0, n1) in ((0, 512), (512, 768)):
                    nc.tensor.matmul(
                        pso[:, n0:n1],
                        attnT01[:, 128 * p:128 * (p + 1)],
                        wo01_sb[:, n0:n1], start=True, stop=False)
                    nc.tensor.matmul(
                        pso[:, n0:n1],
                        attnT2[:, 128 * p:128 * (p + 1)],
                        wo2_sb[:, n0:n1], start=False, stop=True)
                ot2 = ost_pool.tile([128, D], f16, tag="ot")
                if p % 2 == 0:
                    nc.vector.tensor_copy(out=ot2, in_=pso)
                else:
                    nc.scalar.copy(out=ot2, in_=pso)
                t = 2 * p if p < 8 else 2 * (p - 8) + 1
                nc.sync.dma_start(out=out[128 * t:128 * (t + 1), :], in_=ot2)

    nc.compile()
    return nc


def _prep_core_inputs(inputs, c):
    x = inputs["x"]
    Wq, bq = inputs["Wq"], inputs["bq"]
    Wk, bk = inputs["Wk"], inputs["bk"]
    Wv, bv = inputs["Wv"], inputs["bv"]
    Wo = inputs["Wo"]
    b = c // 4
    r0 = (c % 4) * DH  # first feature row of this core's 192-row head block

    xT = np.ascontiguousarray(np.asarray(x[b]).T.astype(bf16))
    W1 = np.ascontiguousarray(np.concatenate(
        [Wq[r0:r0 + 128].T, Wk[r0:r0 + 128].T], axis=1).astype(bf16))
    W2 = np.ascontiguousarray(np.concatenate(
        [Wq[r0 + 128:r0 + 192].T, Wk[r0 + 128:r0 + 192].T], axis=1).astype(bf16))
    Wvp = np.ascontiguousarray(Wv[r0:r0 + 192].T.astype(bf16))
    wo = np.zeros((256, D), bf16)
    wo[0:192] = Wo[:, r0:r0 + 192].T.astype(bf16)

    btCD = np.concatenate([bq[r0 + 128:r0 + 192], bk[r0 + 128:r0 + 192]])
    return dict(
        xT=xT, w1=W1, w2=W2, wv=Wvp, wo=wo,
        btA=np.ascontiguousarray(bq[r0:r0 + 128].reshape(128, 1).astype(np.float32)),
        btB=np.ascontiguousarray(bk[r0:r0 + 128].reshape(128, 1).astype(np.float32)),
        btCD=np.ascontiguousarray(btCD.reshape(128, 1).astype(np.float32)),
        bvb=np.ascontiguousarray(np.tile(
            bv[r0:r0 + 192].reshape(1, 192), (128, 1)).astype(np.float32)),
        stair=np.ascontiguousarray(np.triu(np.ones((128, 128))).astype(bf16)),
        zq=np.zeros((64, S), bf16),
        vinit=_VINIT,
        onesc=np.ones((65, 64), bf16),
    )


def _install_ntff_hook():
    """Register antenv.axon_hooks with a ctypes NTFF profile hook so
    run_bass_kernel_spmd(trace=True) can capture device-side exec time."""
    import types, ctypes, contextlib, importlib

    try:
        import antenv.axon_hooks  # noqa: F401
        return
    except ImportError:
        pass
    so_path = "/opt/axon/libaxon_pjrt.so"
    lib = ctypes.CDLL(so_path)
    if not hasattr(lib, "axon_start_nrt_profile"):
        return
    lib.axon_start_nrt_profile.argtypes = [
        ctypes.POINTER(ctypes.c_int64), ctypes.c_size_t]
    lib.axon_start_nrt_profile.restype = ctypes.c_int64
    lib.axon_stop_nrt_profile.argtypes = [ctypes.c_char_p]
    lib.axon_stop_nrt_profile.restype = ctypes.c_int64

    @contextlib.contextmanager
    def _hook(output_dir, device_ids):
        import jax
        jax.devices()
        if device_ids:
            ids = (ctypes.c_int64 * len(device_ids))(*device_ids)
            rc = lib.axon_start_nrt_profile(ids, len(device_ids))
        else:
            rc = lib.axon_start_nrt_profile(None, 0)
        if rc != 0:
            raise RuntimeError(f"axon_start_nrt_profile rc={rc}")
        try:
            yield
        finally:
            n = lib.axon_stop_nrt_profile(str(output_dir).encode())
            print(f"profile: {n} file(s) written to {output_dir}")

    mod = types.ModuleType("antenv.axon_hooks")
    mod.get_axon_ntff_profile_hook = lambda: _hook
    mod.set_axon_ntff_profile_hook = lambda h: None
    sys.modules["antenv.axon_hooks"] = mod
    import antenv
    antenv.axon_hooks = mod


def kernel(**inputs):
    import os
    from concourse import bass_utils

    if "nc" not in _CACHE:
        _CACHE["nc"] = _build_program()
    nc = _CACHE["nc"]

    trace = bool(os.environ.get("MHA_TRACE"))
    kwargs = {}
    if trace:
        _install_ntff_hook()
        kwargs = dict(trace=True, tmpdir="/tmp/mha_trace")
        os.makedirs("/tmp/mha_trace", exist_ok=True)

    in_maps = [_prep_core_inputs(inputs, c) for c in range(8)]
    res = bass_utils.run_bass_kernel_spmd(
        nc, in_maps, core_ids=list(range(8)), **kwargs)
    _CACHE["last_results"] = res
    if trace and res.exec_time_ns is not None:
        print(f"HW exec time: {res.exec_time_ns} ns")
    out = np.zeros((B, S, D), np.float32)
    for c in range(8):
        out[c // 4] += res.results[c]["out"]
    out += np.asarray(inputs["bo"], np.float32).reshape(1, 1, D)
    return out


# revision 36
# speedup vs baseline: 1.0049x; 1.0049x over previous
"""Trainium2 Bass kernel for nn_MultiHeadAttention_824633721543.

MHA with periodic prefix mask: allowed iff (q % 256) >= (k % 256).
B=2, S=2048, D=768, H=12, Dk=64, WINDOW=256.

Sharding: 8 cores = 2 batches x 4 head-groups (3 heads each). Each core
computes q/k/v projections for its heads, the masked softmax attention, and
a partial O-projection; the host sums the 4 partials per batch and adds bo.

Device-side layout (all transpose-free, fp16 matmul datapath, f32 PSUM):
  - scores computed as S^T [k,q]: kT slice stationary, qT moving
  - q columns tile-permuted (even 128-tiles | odd 128-tiles) so the mask is:
      even-group x k-lo  -> one shared 128x128 triu tile (0/1 multiply)
      odd-group  x k-lo  -> unmasked
      odd-group  x k-hi  -> shared triu
      even-group x k-hi  -> fully masked, never computed
  - exp on ACT (scale=1/8 folded in, f32 psum in -> fp16 out), no
    max-subtraction (scores are small)
  - P@V with [V|1] stationary -> out^T plus denominator row, accumulated over
    the 8 windows in PSUM; normalization: reciprocal of the denom row,
    K=1 broadcast matmul, DVE multiply
  - O-projection: h0+h1 stacked into one K=128 stationary + h2 at K=64;
    result DMA'd to DRAM directly from PSUM (f32)
"""

import sys

sys.path.insert(0, "/opt/trn_rl_repo")

import numpy as np
import ml_dtypes

bf16 = np.float16

B = 2
S = 2048
D = 768
DK = 64
WIN = 256
NW = S // WIN   # 8 windows
NHC = 3         # heads per core
DH = NHC * DK   # 192
NT = S // 128   # 16 q tiles

_CACHE = {}

# v-tile init pattern: three 128-col groups, ones at col 64 of each group
_VINIT = np.zeros((128, 384), np.float16)
_VINIT[:, [64, 192, 320]] = 1.0


def _build_program():
    import concourse.tile as tile
    from concourse import mybir, bacc
    from contextlib import ExitStack

    f32 = mybir.dt.float32
    f16 = mybir.dt.float16
    Exp = mybir.ActivationFunctionType.Exp
    Ident = mybir.ActivationFunctionType.Identity
    mult = mybir.AluOpType.mult
    add = mybir.AluOpType.add

    nc = bacc.Bacc("TRN2", target_bir_lowering=False, debug=False)

    xT = nc.dram_tensor("xT", [D, S], f16, kind="ExternalInput").ap()
    w1 = nc.dram_tensor("w1", [D, 256], f16, kind="ExternalInput").ap()  # [qh01|kh01]
    w2 = nc.dram_tensor("w2", [D, 128], f16, kind="ExternalInput").ap()  # [qh2|kh2]
    wv = nc.dram_tensor("wv", [D, 192], f16, kind="ExternalInput").ap()  # WvT
    wo = nc.dram_tensor("wo", [256, D], f16, kind="ExternalInput").ap()
    btA = nc.dram_tensor("btA", [128, 1], f32, kind="ExternalInput").ap()
    btB = nc.dram_tensor("btB", [128, 1], f32, kind="ExternalInput").ap()
    btCD = nc.dram_tensor("btCD", [128, 1], f32, kind="ExternalInput").ap()
    bvb = nc.dram_tensor("bvb", [128, 192], f32, kind="ExternalInput").ap()
    stair = nc.dram_tensor("stair", [128, 128], f16, kind="ExternalInput").ap()
    zq = nc.dram_tensor("zq", [64, S], f16, kind="ExternalInput").ap()
    vinit = nc.dram_tensor("vinit", [128, 384], f16, kind="ExternalInput").ap()
    onesc = nc.dram_tensor("onesc", [65, 64], f16, kind="ExternalInput").ap()
    out = nc.dram_tensor("out", [S, D], f16, kind="ExternalOutput").ap()
    import os as _os
    DBG = bool(_os.environ.get("MHA_DEBUG"))
    if DBG:
        dbgA = nc.dram_tensor("dbgA", [128, S], f16, kind="ExternalOutput").ap()
        dbg2 = nc.dram_tensor("dbg2", [64, S], f16, kind="ExternalOutput").ap()
        dbgden = nc.dram_tensor("dbgden", [192, S], f32, kind="ExternalOutput").ap()

    with tile.TileContext(nc) as tc, ExitStack() as ctx:
        consts = ctx.enter_context(tc.tile_pool(name="consts", bufs=1))
        qkv = ctx.enter_context(tc.tile_pool(name="qkv", bufs=1))

        xT_sb = [qkv.tile([128, S], f16, tag=f"xt{k}", name=f"xt{k}")
                 for k in range(6)]
        w1_sb = [consts.tile([128, 256], f16, tag=f"w1_{k}", name=f"w1s{k}")
                 for k in range(6)]
        w2_sb = [consts.tile([128, 128], f16, tag=f"w2_{k}", name=f"w2s{k}")
                 for k in range(6)]
        wv_sb = [consts.tile([128, 192], f16, tag=f"wv_{k}", name=f"wvs{k}")
                 for k in range(6)]
        for k in range(6):
            nc.sync.dma_start(out=xT_sb[k], in_=xT[k * 128:(k + 1) * 128, :])
            nc.sync.dma_start(out=w1_sb[k], in_=w1[k * 128:(k + 1) * 128, :])
            nc.sync.dma_start(out=w2_sb[k], in_=w2[k * 128:(k + 1) * 128, :])
            nc.sync.dma_start(out=wv_sb[k], in_=wv[k * 128:(k + 1) * 128, :])
        # Wo^T: h0+h1 as one 128-row tile, h2 zero-padded to 128 rows
        wo01_sb = consts.tile([128, D], f16, tag="wo01")
        wo2_sb = consts.tile([128, D], f16, tag="wo2")
        nc.sync.dma_start(out=wo01_sb, in_=wo[0:128, :])
        nc.sync.dma_start(out=wo2_sb, in_=wo[128:256, :])
        btA_sb = consts.tile([128, 1], f32, tag="btA")
        btB_sb = consts.tile([128, 1], f32, tag="btB")
        btCD_sb = consts.tile([128, 1], f32, tag="btCD")
        nc.sync.dma_start(out=btA_sb, in_=btA)
        nc.sync.dma_start(out=btB_sb, in_=btB)
        nc.sync.dma_start(out=btCD_sb, in_=btCD)
        bvb_sb = consts.tile([128, 192], f32, tag="bvb")
        nc.sync.dma_start(out=bvb_sb, in_=bvb)
        stair_sb = consts.tile([128, 128], f16, tag="stair")
        nc.sync.dma_start(out=stair_sb, in_=stair)
        ones_row = consts.tile([65, 64], f16, tag="ones_row")
        nc.sync.dma_start(out=ones_row, in_=onesc)

        # ---- long-lived activation tiles ----
        # Per-head qT/kT tiles [128, S]: K=128 stationaries for the PE.
        # h0/h2 data lives in rows 0:64, h1/kh2 in rows 64:128; the other
        # 64 rows are zero so the padded contraction adds nothing.
        qh = [qkv.tile([128, S], f16, tag=f"qh{i}", name=f"qh{i}")
              for i in range(3)]
        kh = [qkv.tile([128, S], f16, tag=f"kh{i}", name=f"kh{i}")
              for i in range(3)]
        qrow = [0, 64, 64]  # base row of the 64 data rows per head
        krow = [0, 64, 64]
        for i in range(3):
            zr = 64 - qrow[i]
            nc.sync.dma_start(out=qh[i][zr:zr + 64, :], in_=zq)
            zr = 64 - krow[i]
            nc.sync.dma_start(out=kh[i][zr:zr + 64, :], in_=zq)
        # v natural [s,d] per s-tile: three 128-col groups [V_h | 1 | 0...]
        v_sb = [qkv.tile([128, 384], f16, tag=f"v{i}", name=f"vsb{i}")
                for i in range(NT)]
        for st in range(NT):
            nc.sync.dma_start(out=v_sb[st], in_=vinit)
        # attn^T: h0 rows 0:64 + h1 rows 64:128 in one tile; h2 zero-padded
        attnT01 = qkv.tile([128, S], f16, tag="attnT01")
        attnT2 = qkv.tile([128, S], f16, tag="attnT2")
        nc.sync.dma_start(out=attnT2[64:128, :], in_=zq)

        def permuted_copy(dst, r0, ps, n, bias, eng, src_r0=None):
            """psum rows src_r0:+64, 512-span n -> dst rows r0:r0+64 with
            even/odd q-tile permutation."""
            s0 = r0 if src_r0 is None else src_r0
            pr3 = ps[s0:s0 + 64, :].rearrange(
                "p (c two k) -> p c two k", two=2, k=128)
            dr = dst[r0:r0 + 64, :]
            bs = bias[s0:s0 + 64, :]
            eng.tensor_scalar_add(
                out=dr[:, 256 * n:256 * n + 256].rearrange("p (c k) -> p c k", k=128),
                in0=pr3[:, :, 0, :], scalar1=bs)
            eng.tensor_scalar_add(
                out=dr[:, 1024 + 256 * n:1024 + 256 * n + 256].rearrange(
                    "p (c k) -> p c k", k=128),
                in0=pr3[:, :, 1, :], scalar1=bs)

        # ---- stage A ----
        with tc.tile_pool(name="psA", bufs=2, space="PSUM") as psA:
            for n in range(4):
                xn = [xT_sb[k][:, 512 * n:512 * (n + 1)]
                      for k in range(6)]
                psa = psA.tile([128, 512], f32, tag="psA")
                for k in range(6):
                    nc.tensor.matmul(psa, w1_sb[k][:, 0:128], xn[k],
                                     start=(k == 0), stop=(k == 5))
                permuted_copy(qh[0], 0, psa, n, btA_sb, nc.vector)
                permuted_copy(qh[1], 64, psa, n, btA_sb, nc.vector)
                psb = psA.tile([128, 512], f32, tag="psA")
                for k in range(6):
                    nc.tensor.matmul(psb, w1_sb[k][:, 128:256], xn[k],
                                     start=(k == 0), stop=(k == 5))
                nc.scalar.activation(
                    out=kh[0][0:64, 512 * n:512 * (n + 1)], in_=psb[0:64, :],
                    func=Ident, bias=btB_sb[0:64, :])
                nc.scalar.activation(
                    out=kh[1][64:128, 512 * n:512 * (n + 1)], in_=psb[64:128, :],
                    func=Ident, bias=btB_sb[64:128, :])
                psqk = psA.tile([128, 512], f32, tag="psA")
                for k in range(6):
                    nc.tensor.matmul(psqk, w2_sb[k], xn[k],
                                     start=(k == 0), stop=(k == 5))
                permuted_copy(qh[2], 64, psqk, n, btCD_sb, nc.vector,
                              src_r0=0)
                nc.scalar.activation(
                    out=kh[2][64:128, 512 * n:512 * (n + 1)], in_=psqk[64:128, :],
                    func=Ident, bias=btCD_sb[64:128, :])


        # ---- stage B ----
        heads = [
            dict(q=qh[0], k=kh[0], o=(attnT01, 0)),
            dict(q=qh[1], k=kh[1], o=(attnT01, 64)),
            dict(q=qh[2], k=kh[2], o=(attnT2, 0)),
        ]

        stair_b = stair_sb.unsqueeze(1).broadcast_to([128, 8, 128])
        with tc.tile_pool(name="pt", bufs=8) as pt_pool, \
             tc.tile_pool(name="sc", bufs=3, space="PSUM") as sc_pool, \
             tc.tile_pool(name="po", bufs=2, space="PSUM") as out_pool, \
             tc.tile_pool(name="psv", bufs=1, space="PSUM") as psv_pool, \
             tc.tile_pool(name="nrm", bufs=2) as nrm_pool:
            def v_proj(st):
                psv = psv_pool.tile([128, 192], f32, tag="psv")
                for k in range(6):
                    nc.tensor.matmul(
                        psv, xT_sb[k][:, 128 * st:128 * (st + 1)],
                        wv_sb[k], start=(k == 0), stop=(k == 5))
                vt = v_sb[st]
                nc.vector.tensor_tensor(
                    out=vt.rearrange("p (h c) -> p h c", c=128)[:, :, 0:64],
                    in0=psv.rearrange("p (h c) -> p h c", c=64),
                    in1=bvb_sb.rearrange("p (h c) -> p h c", c=64), op=add)

            for h in range(NHC):
                hd = heads[h]
                qv = hd["q"]
                kv = hd["k"]
                ot, ooff = hd["o"]
                vh = [v[:, 128 * h:128 * (h + 1)] for v in v_sb]  # [V_h|1|0]

                po = [out_pool.tile([128, 1024], f32, tag="po",
                                    name=f"po{h}_{g}") for g in range(2)]
                first = [[True, True], [True, True]]

                def pv_mm(g, vsl, pt, last):
                    for sub in range(2):
                        nc.tensor.matmul(
                            po[g][:, 512 * sub:512 * (sub + 1)],
                            vsl,
                            pt[:, 512 * sub:512 * (sub + 1)],
                            start=first[g][sub], stop=last)
                        first[g][sub] = False

                def scores_exp(g, kblk, mask):
                    qcols = qv[:, 1024 * g:1024 * (g + 1)]
                    pt = pt_pool.tile([128, 1024], f16, tag="pt")
                    for sub in range(2):
                        sch = sc_pool.tile([128, 512], f32, tag="sc")
                        nc.tensor.matmul(
                            sch, kblk, qcols[:, 512 * sub:512 * (sub + 1)],
                            start=True, stop=True)
                        nc.scalar.activation(
                            out=pt[:, 512 * sub:512 * (sub + 1)], in_=sch,
                            func=Exp, scale=0.125)
                    if mask:
                        p3 = pt.rearrange("p (c k) -> p c k", k=128)
                        nc.vector.tensor_mul(out=p3, in0=p3, in1=stair_b)
                    return pt

                # g0/g1 interleaved, PV one window behind scores
                pend = None
                for w in range(NW + 1):
                    if w < NW:
                        if h == 0:
                            v_proj(2 * w)
                            v_proj(2 * w + 1)
                        klo = kv[:, WIN * w:WIN * w + 128]
                        khi = kv[:, WIN * w + 128:WIN * w + 256]
                        cur = (scores_exp(0, klo, True),
                               scores_exp(1, klo, False),
                               scores_exp(1, khi, True))
                    if pend is not None:
                        pw = w - 1
                        last = (pw == NW - 1)
                        pv_mm(0, vh[2 * pw], pend[0], last)
                        pv_mm(1, vh[2 * pw], pend[1], False)
                        pv_mm(1, vh[2 * pw + 1], pend[2], last)
                    pend = cur if w < NW else None

                # normalization: reciprocal of denom row 64 -> bcast -> mul
                for g in range(2):
                    poc = nrm_pool.tile([65, 1024], f16, tag="poc",
                                        name=f"poc{h}{g}")
                    nc.vector.tensor_copy(out=poc, in_=po[g][0:65, :])
                    rec_sb = nrm_pool.tile([64, 1024], f32, tag="rec")
                    for sub in range(2):
                        rec_ps = sc_pool.tile([128, 512], f32, tag="sc")
                        nc.tensor.matmul(
                            rec_ps[0:64, :],
                            ones_row[64:65, :],
                            poc[64:65, 512 * sub:512 * (sub + 1)],
                            start=True, stop=True)
                        nc.vector.reciprocal_approx_fast(
                            out=rec_sb[:, 512 * sub:512 * (sub + 1)],
                            in_=rec_ps[0:64, :])
                    nc.vector.tensor_tensor(
                        out=ot[ooff:ooff + 64, 1024 * g:1024 * (g + 1)],
                        in0=poc[0:64, :], in1=rec_sb, op=mult)
                    if DBG:
                        dent = nrm_pool.tile([128, 1024], f32, tag=f"dent{h}{g}",
                                             name=f"dent{h}{g}")
                        nc.vector.tensor_copy(out=dent[0:64, :], in_=rec_ps[0:64, :])
                        nc.sync.dma_start(
                            out=dbgden[64 * h:64 * (h + 1), 1024 * g:1024 * (g + 1)],
                            in_=dent[0:64, :])

        if DBG:
            nc.sync.dma_start(out=dbgA, in_=attnT01)
            nc.sync.dma_start(out=dbg2, in_=attnT2)

        # ---- stage C ----
        with tc.tile_pool(name="oc", bufs=4, space="PSUM") as oc_pool, \
             tc.tile_pool(name="ost", bufs=4) as ost_pool:
            for p in range(NT):
                pso = oc_pool.tile([128, D], f32, tag="pso")
                for (n0, n1) in ((0, 512), (512, 768)):
                    nc.tensor.matmul(
                        pso[:, n0:n1],
                        attnT01[:, 128 * p:128 * (p + 1)],
                        wo01_sb[:, n0:n1], start=True, stop=False)
                    nc.tensor.matmul(
                        pso[:, n0:n1],
                        attnT2[:, 128 * p:128 * (p + 1)],
                        wo2_sb[:, n0:n1], start=False, stop=True)
                ot2 = ost_pool.tile([128, D], f16, tag="ot")
                if p % 2 == 0:
                    nc.vector.tensor_copy(out=ot2, in_=pso)
                else:
                    nc.scalar.copy(out=ot2, in_=pso)
                t = 2 * p if p < 8 else 2 * (p - 8) + 1
                nc.sync.dma_start(out=out[128 * t:128 * (t + 1), :], in_=ot2)

    nc.compile()
    return nc


def _prep_core_inputs(inputs, c):
    x = inputs["x"]
    Wq, bq = inputs["Wq"], inputs["bq"]
    Wk, bk = inputs["Wk"], inputs["bk"]
    Wv, bv = inputs["Wv"], inputs["bv"]
    Wo = inputs["Wo"]
    b = c // 4
    r0 = (c % 4) * DH  # first feature row of this core's 192-row head block

    xT = np.ascontiguousarray(np.asarray(x[b]).T.astype(bf16))
    W1 = np.ascontiguousarray(np.concatenate(
        [Wq[r0:r0 + 128].T, Wk[r0:r0 + 128].T], axis=1).astype(bf16))
    W2 = np.ascontiguousarray(np.concatenate(
        [Wq[r0 + 128:r0 + 192].T, Wk[r0 + 128:r0 + 192].T], axis=1).astype(bf16))
    Wvp = np.ascontiguousarray(Wv[r0:r0 + 192].T.astype(bf16))
    wo = np.zeros((256, D), bf16)
    wo[0:192] = Wo[:, r0:r0 + 192].T.astype(bf16)

    btCD = np.concatenate([bq[r0 + 128:r0 + 192], bk[r0 + 128:r0 + 192]])
    return dict(
        xT=xT, w1=W1, w2=W2, wv=Wvp, wo=wo,
        btA=np.ascontiguousarray(bq[r0:r0 + 128].reshape(128, 1).astype(np.float32)),
        btB=np.ascontiguousarray(bk[r0:r0 + 128].reshape(128, 1).astype(np.float32)),
        btCD=np.ascontiguousarray(btCD.reshape(128, 1).astype(np.float32)),
        bvb=np.ascontiguousarray(np.tile(
            bv[r0:r0 + 192].reshape(1, 192), (128, 1)).astype(np.float32)),
        stair=np.ascontiguousarray(np.triu(np.ones((128, 128))).astype(bf16)),
        zq=np.zeros((64, S), bf16),
        vinit=_VINIT,
        onesc=np.ones((65, 64), bf16),
    )


def _install_ntff_hook():
    """Register antenv.axon_hooks with a ctypes NTFF profile hook so
    run_bass_kernel_spmd(trace=True) can capture device-side exec time."""
    import types, ctypes, contextlib, importlib

    try:
        import antenv.axon_hooks  # noqa: F401
        return
    except ImportError:
        pass
    so_path = "/opt/axon/libaxon_pjrt.so"
    lib = ctypes.CDLL(so_path)
    if not hasattr(lib, "axon_start_nrt_profile"):
        return
    lib.axon_start_nrt_profile.argtypes = [
        ctypes.POINTER(ctypes.c_int64), ctypes.c_size_t]
    lib.axon_start_nrt_profile.restype = ctypes.c_int64
    lib.axon_stop_nrt_profile.argtypes = [ctypes.c_char_p]
    lib.axon_stop_nrt_profile.restype = ctypes.c_int64

    @contextlib.contextmanager
    def _hook(output_dir, device_ids):
        import jax
        jax.devices()
        if device_ids:
            ids = (ctypes.c_int64 * len(device_ids))(*device_ids)
            rc = lib.axon_start_nrt_profile(ids, len(device_ids))
        else:
            rc = lib.axon_start_nrt_profile(None, 0)
        if rc != 0:
            raise RuntimeError(f"axon_start_nrt_profile rc={rc}")
        try:
            yield
        finally:
            n = lib.axon_stop_nrt_profile(str(output_dir).encode())
            print(f"profile: {n} file(s) written to {output_dir}")

    mod = types.ModuleType("antenv.axon_hooks")
    mod.get_axon_ntff_profile_hook = lambda: _hook
    mod.set_axon_ntff_profile_hook = lambda h: None
    sys.modules["antenv.axon_hooks"] = mod
    import antenv
    antenv.axon_hooks = mod


def kernel(**inputs):
    import os
    from concourse import bass_utils

    if "nc" not in _CACHE:
        _CACHE["nc"] = _build_program()
    nc = _CACHE["nc"]

    trace = bool(os.environ.get("MHA_TRACE"))
    kwargs = {}
    if trace:
        _install_ntff_hook()
        kwargs = dict(trace=True, tmpdir="/tmp/mha_trace")
        os.makedirs("/tmp/mha_trace", exist_ok=True)

    in_maps = [_prep_core_inputs(inputs, c) for c in range(8)]
    res = bass_utils.run_bass_kernel_spmd(
        nc, in_maps, core_ids=list(range(8)), **kwargs)
    _CACHE["last_results"] = res
    if trace and res.exec_time_ns is not None:
        print(f"HW exec time: {res.exec_time_ns} ns")
    out = np.zeros((B, S, D), np.float32)
    for c in range(8):
        out[c // 4] += res.results[c]["out"]
    out += np.asarray(inputs["bo"], np.float32).reshape(1, 1, D)
    return out
